# revision 22
# baseline (speedup 1.0000x reference)
"""Trainium2 Bass kernel for nn_CentersDistance (retrieval_knn).

logits[k, n] = -||centers[k] - inputs[n]||^2
             = 2*(centers @ inputs.T)[k, n] - ||centers[k]||^2 - ||inputs[n]||^2

Strategy (8 NeuronCores, data-parallel over the N=8192 inputs):
  * host: transpose both operands so the contraction dim D lands on the SBUF
    partition axis, fold the factor 2 into the inputs, quantize both to
    fp8e4m3 (TRN float8e4), and precompute the norm terms exactly in float64.
  * device (per core): a 1024x1024x1024 matmul in fp8 with DoubleRow perf
    mode: each InstMatmult consumes TWO 128-deep contraction tiles laid out
    as [128, 2, free] (2 rows/cycle on the PE = 157 TF/s, 2x the bf16 rate),
    so the whole GEMM is 64 matmul instructions instead of 128.  PSUM
    accumulation stays fp32.
  * the PSUM->SBUF epilogue (add -||c||^2 per-partition and -||x||^2
    broadcast row, emit bf16) is the serial tail bottleneck (~740ns per
    [128, 512] group on the DVE), so it is split across THREE engines:
      - even groups: DVE scalar_tensor_tensor (one op per group);
      - odd groups: Activation engine adds the per-partition -||c||^2 via
        an Identity-activation bias read straight from PSUM (this also
        frees the PSUM bank for pass 2), then GpSimd adds the -||x||^2 row
        (tensor_add, SBUF-only - GpSimd cannot touch PSUM).
  * -||x||^2 is shipped as a single [1, 1024] fp32 row (4 KB) and broadcast
    to all 128 partitions on-chip by GpSimd (partition_broadcast), replacing
    the baseline's 512 KB host-broadcast load.
  * raw Block/semaphore implementation (not Tile), same skeleton as the
    bf16 baseline: two HW-DGE queues (Sync: xt, Scalar: ct) with one
    semaphore per d-pair; PE warmup matmuls bridge the NRT preamble until
    the first tile pair lands (~10.5us) and keep the HAM clock ramp alive
    (an idle PE resets it - observed: a 4us tile stall restarted the ramp
    and cost 13 matmuls at 1.2GHz); pass 1 (m-tiles 0-3) runs d outermost
    to pace with the streaming loads across 8 PSUM banks; pass 2 (m-tiles
    4-7) runs d innermost so each output group retires early and its
    epilogue + store overlap the remaining matmuls.
  * stores pair adjacent groups (same m-tile -> contiguous in out) into
    single [128, 1024] bf16 DMAs (2 KB/partition lines) alternating between
    the two queues; the last group is split in half across both queues to
    shorten the tail.  Output is bf16 (halves store traffic vs fp32; host
    converts back).

Accuracy: the exact f64 norm terms dominate the logits; fp8 cross term +
bf16 store measured absmax/scale 7.3e-3 vs the 2e-2 gate (bf16 baseline:
3.3e-4 at 45us, kept in kernel_bf16_baseline.py as fallback).

Measured on 8 axon-tunneled trn2 cores (NTFF): 30.5-32.9us across runs
(best 30490ns, median ~31.4us); shared-HBM/clock-throttle noise from
co-tenants can add up to +7us in bad windows.  Accuracy: absmax/scale
5.22e-3 vs the 2e-2 gate, bit-stable (fixed-seed inputs, deterministic
quantization and accumulation order).  bf16 fallback (3.3e-4, 45us) in
kernel_bf16_baseline.py.
"""

import threading
from contextlib import ExitStack

import numpy as np
import ml_dtypes

import concourse.mybir as mybir
from concourse import bacc
from concourse.bass_utils import run_bass_kernel_spmd

N_CORES = 8
N, K, D = 8192, 1024, 1024
NSH = N // N_CORES  # per-core slab of inputs
P = 128             # SBUF partitions
NF = 512            # matmul moving free dim (one fp32 PSUM bank)

DP_TILES = D // (2 * P)  # 4 double-row contraction tiles (256 deep each)
M_TILES = K // P         # 8 center tiles
H_TILES = NSH // NF      # 2 moving-dim tiles

G = M_TILES * H_TILES  # 16 output groups of [128, 512]
GP1 = 8                # groups 0-7 -> pass 1 (m-tiles 0-3), banks 0-7
N_WU = 10              # PE warm-up matmuls (bridge preamble-end -> first tile)
N_TMP = 6              # fp16 staging buffers for the Act->DVE drain pipeline

_DT = mybir.dt.float8e4
_NP_DT = ml_dtypes.float8_e4m3
_OUT_DT = mybir.dt.float16
_DR = mybir.MatmulPerfMode.DoubleRow

_cache = threading.local()


def _g_mh(g):
    return g // H_TILES, g % H_TILES


def _build_nc():
    nc = bacc.Bacc(
        "TRN2", target_bir_lowering=False, debug=False, num_devices=N_CORES
    )
    ct = nc.dram_tensor("ct", [DP_TILES, P, 2, K], _DT, kind="ExternalInput").ap()
    xt = nc.dram_tensor("xt", [DP_TILES, P, 2, NSH], _DT, kind="ExternalInput").ap()
    ncsq = nc.dram_tensor(
        "ncsq", [P, M_TILES], mybir.dt.float32, kind="ExternalInput"
    ).ap()
    nxsq = nc.dram_tensor(
        "nxsq", [P, NSH], mybir.dt.float16, kind="ExternalInput"
    ).ap()
    out = nc.dram_tensor("out", [K, NSH], _OUT_DT, kind="ExternalOutput").ap()

    out_r = out.rearrange("(m p) n -> m p n", p=P)

    HNF = NF // 2

    # Epilogue schedule.  DIRECT groups: one DVE scalar_tensor_tensor from
    # PSUM.  Others: Act drains (ps + ncsq -> tmp fp16), DVE finishes
    # (tmp + nxsq -> ot).  Group 15 is processed in two halves for tail
    # latency.  DVE processes groups in order, so od_sem == number of
    # groups (items) whose ot_sb data is final - stores key off it.
    DIRECT = {0, 4, G - 2, G - 1}
    items = [(g, 0, NF) for g in range(G - 1)]
    items += [(G - 1, 0, HNF), (G - 1, HNF, NF)]
    DVE_LIST = items
    ACT_LIST = [it for it in items if it[0] not in DIRECT]

    def _item_glh(item):
        return item

    OB = {}   # (g, lo) -> ob_sem count after its Act drain
    BUF = {}  # (g, lo) -> tmp buffer index
    for j, (g, lo, hi) in enumerate(ACT_LIST):
        OB[(g, lo)] = j + 1
        BUF[(g, lo)] = j % N_TMP
    FIN_OD = {}  # item -> od count after its DVE finish
    for j, it in enumerate(DVE_LIST):
        FIN_OD[it] = j + 1
    # pass-2 bank reuse: who releases bank g (the engine that READS PSUM)
    DRAIN_DONE = {}
    for g in range(GP1):
        if g in DIRECT:
            DRAIN_DONE[g] = ("od", FIN_OD[(g, 0, NF)])
        else:
            DRAIN_DONE[g] = ("ob", OB[(g, 0)])

    with (
        nc.sbuf_tensor("wu_sb", [P, 2, NF], _DT) as wu_sb,
        nc.sbuf_tensor("ncsq_sb", [P, M_TILES], mybir.dt.float32) as ncsq_sb,
        nc.sbuf_tensor("nxsq_sb", [P, NSH], mybir.dt.float16) as nxsq_sb,
        nc.sbuf_tensor("tmp_sb", [P, N_TMP, NF], mybir.dt.float16) as tmp_sb,
        nc.sbuf_tensor("ot_sb", [P, G * NF], _OUT_DT) as ot_sb,
        ExitStack() as stack,
        nc.semaphore("const_sem") as const_sem,
        nc.semaphore("bc_sem") as bc_sem,
        nc.semaphore("mm_sem") as mm_sem,
        nc.semaphore("ob_sem") as ob_sem, # group PSUM drained (Act)
        nc.semaphore("od_sem") as od_sem, # group data ready in ot_sb (DVE)
        nc.semaphore("ds_sync") as ds_sync,
        nc.semaphore("ds_ring") as ds_ring,
        nc.semaphore("ds_scalar") as ds_scalar,
        nc.Block() as block,
    ):
        d_sems = [
            stack.enter_context(nc.semaphore(f"d_sem{i}")) for i in range(DP_TILES)
        ]
        ct_sb = [
            stack.enter_context(nc.sbuf_tensor(f"ct_sb{d}", [P, 2, K], _DT))
            for d in range(DP_TILES)
        ]
        xt_sb = [
            stack.enter_context(nc.sbuf_tensor(f"xt_sb{d}", [P, 2, NSH], _DT))
            for d in range(DP_TILES)
        ]
        ps = [
            stack.enter_context(nc.psum_tensor(f"ps{b}", [P, NF], mybir.dt.float32))
            for b in range(8)
        ]

        # store schedule: pair adjacent groups (same m-tile -> contiguous
        # in out) into one [128, 1024] fp16 DMA with 2KB/partition lines.
        # Sync queue: pairs 0,2,4,6 + group 14 + first half of group 15.
        # GpSimd SWDGE ring: pairs 1,3,5 + second half of group 15 (the Act
        # engine dispatches no stores at all - it is the single PSUM
        # drainer and must not stall behind store waits).
        def pair_store(eng, gp, sem):
            g0 = 2 * gp
            m, _ = _g_mh(g0)
            eng.wait_ge(od_sem, g0 + 2)
            eng.dma_start(
                out_r[m][:],
                ot_sb[:, g0 * NF : (g0 + 2) * NF],
            ).then_inc(sem, 16)

        @block.sync
        def _(sync):
            # ncsq first: 4KB, needed by the Act drains
            sync.dma_start(ncsq_sb[:], ncsq).then_inc(const_sem, 16)
            for d in (0, 2, 3):
                sync.dma_start(xt_sb[d][:], xt[d]).then_inc(d_sems[d], 16)
            for gp in (0, 2, 4):
                pair_store(sync, gp, ds_sync)
            # group 14 whole
            m, h = _g_mh(G - 2)
            sync.wait_ge(od_sem, G - 1)
            sync.dma_start(
                out_r[m][:, h * NF : (h + 1) * NF],
                ot_sb[:, (G - 2) * NF : (G - 1) * NF],
            ).then_inc(ds_sync, 16)
            # first half of group 15
            m, h = _g_mh(G - 1)
            sync.wait_ge(od_sem, G)
            sync.dma_start(
                out_r[m][:, h * NF : h * NF + HNF],
                ot_sb[:, (G - 1) * NF : (G - 1) * NF + HNF],
            ).then_inc(ds_sync, 16)
            # wait for THIS queue's stores only; scalar and gpsimd wait for
            # theirs in parallel, so the end-of-kernel barrier sees the max,
            # not the sum of completion-semaphore latencies
            sync.wait_ge(ds_sync, 5 * 16)

        @block.scalar
        def _(scalar):
            for d in (0, 2, 3):
                scalar.dma_start(ct_sb[d][:], ct[d]).then_inc(d_sems[d], 16)
            # Act drains the non-DIRECT groups: tmp = Identity(ps +
            # ncsq_bias), fp32 PSUM -> fp16 SBUF, ~720ns measured per
            # [128, 512] group.  DIRECT groups go via a single DVE
            # scalar_tensor_tensor instead (745ns) - the 4/12 split
            # balances the two engines at ~8.5us each.
            scalar.wait_ge(const_sem, 16)
            n_act = 0

            def act_drain(g, lo, hi):
                nonlocal n_act
                buf = n_act % N_TMP
                if n_act >= N_TMP:
                    # tmp reuse: the DVE finish for this buffer's previous
                    # occupant must have consumed it
                    scalar.wait_ge(od_sem, FIN_OD[ACT_LIST[n_act - N_TMP]])
                m, _ = _g_mh(g)
                nc.scalar.add(
                    tmp_sb[:, buf, 0 : hi - lo],
                    ps[g % 8][:, lo:hi],
                    ncsq_sb[:, m : m + 1],
                ).then_inc(ob_sem, 1)
                n_act += 1

            for item in ACT_LIST:
                g, lo, hi = _item_glh(item)
                scalar.wait_ge(mm_sem, min(g + 1, G))
                act_drain(g, lo, hi)
            # late stores from here: the Act engine is idle after its last
            # drain and dispatches fast
            pair_store(scalar, 6, ds_scalar)
            m, h = _g_mh(G - 1)
            scalar.wait_ge(od_sem, G + 1)
            scalar.dma_start(
                out_r[m][:, h * NF + HNF : (h + 1) * NF],
                ot_sb[:, (G - 1) * NF + HNF : G * NF],
            ).then_inc(ds_scalar, 16)
            scalar.wait_ge(ds_scalar, 2 * 16)

        @block.gpsimd
        def _(gpsimd):
            # dp1 rides the software-DGE ring: it spins up ~2us after the HW
            # queues and lands its pair second, right when pass 1 wants it.
            # GpSimd does DMA dispatch ONLY (its partition_broadcast ucode
            # carries a pool-reconfig DRAIN that blocked the engine ~12us),
            # so nxsq ships host-broadcast (256KB fp16) as the third stream.
            gpsimd.dma_start(xt_sb[1][:], xt[1]).then_inc(d_sems[1], 16)
            gpsimd.dma_start(ct_sb[1][:], ct[1]).then_inc(d_sems[1], 16)
            gpsimd.dma_start(nxsq_sb[:], nxsq).then_inc(bc_sem, 16)
            # odd pair stores ride the ring too
            for gp in (1, 3, 5):
                pair_store(gpsimd, gp, ds_ring)
            gpsimd.wait_ge(ds_ring, 3 * 16)

        @block.tensor
        def _(tensor):
            # warm-up: keep the PE busy (and the HAM clock ramp alive) from
            # preamble-end until the first ct/xt tile pair lands.  wu_sb is
            # deliberately uninitialized - the products are never read.
            # Bank 7 is rewritten with start=True by group 7's first matmul
            # ~8 matmuls later.
            for _ in range(N_WU):
                nc.tensor.matmul(
                    ps[GP1 - 1][:],
                    wu_sb[:, :, 0:P],
                    wu_sb[:, :, :],
                    start=True,
                    stop=True,
                    perf_mode=_DR,
                )
            # pass 1: groups 0-7 accumulate in banks 0-7, d outermost so
            # matmuls pace with the streaming loads
            for d in range(DP_TILES):
                tensor.wait_ge(d_sems[d], 32)
                for g in range(GP1):
                    m, h = _g_mh(g)
                    mm = nc.tensor.matmul(
                        ps[g][:],
                        ct_sb[d][:, :, m * P : (m + 1) * P],
                        xt_sb[d][:, :, h * NF : (h + 1) * NF],
                        start=(d == 0),
                        stop=(d == DP_TILES - 1),
                        perf_mode=_DR,
                    )
                    if d == DP_TILES - 1:
                        mm.then_inc(mm_sem, 1)
            # pass 2: groups 8-15 reuse banks 0-7 once the Act engine has
            # drained the pass-1 group from that bank (P10: concurrent
            # PE-write + engine-read of one PSUM bank is fatal, so this wait
            # is load-bearing, not just WAR ordering)
            for g in range(GP1, G):
                m, h = _g_mh(g)
                dsem, dcnt = DRAIN_DONE[g - 8]
                tensor.wait_ge(ob_sem if dsem == "ob" else od_sem, dcnt)
                for d in range(DP_TILES):
                    mm = nc.tensor.matmul(
                        ps[g % 8][:],
                        ct_sb[d][:, :, m * P : (m + 1) * P],
                        xt_sb[d][:, :, h * NF : (h + 1) * NF],
                        start=(d == 0),
                        stop=(d == DP_TILES - 1),
                        perf_mode=_DR,
                    )
                mm.then_inc(mm_sem, 1)

        @block.vector
        def _(vector):
            vector.wait_ge(bc_sem, 16)     # nxsq landed
            vector.wait_ge(const_sem, 16)  # ncsq (for the direct STTs)

            def tt_finish(g, lo, hi, buf):
                # finish: ot = tmp + nxsq.  All-fp16 SBUF operands -> DVE
                # 2x perf mode (~423ns vs 745ns for the PSUM STT).
                vector.wait_ge(ob_sem, OB[(g, lo)])
                _, h = _g_mh(g)
                nc.vector.tensor_add(
                    ot_sb[:, g * NF + lo : g * NF + hi],
                    tmp_sb[:, buf, 0 : hi - lo],
                    nxsq_sb[:, h * NF + lo : h * NF + hi],
                ).then_inc(od_sem, 1)

            for item in DVE_LIST:
                g, lo, hi = _item_glh(item)
                if g in DIRECT:
                    m, h = _g_mh(g)
                    vector.wait_ge(mm_sem, min(g + 1, G))
                    nc.vector.scalar_tensor_tensor(
                        ot_sb[:, g * NF + lo : g * NF + hi],
                        ps[g % 8][:, lo:hi],
                        ncsq_sb[:, m : m + 1],
                        nxsq_sb[:, h * NF + lo : h * NF + hi],
                        op0=mybir.AluOpType.add,
                        op1=mybir.AluOpType.add,
                    ).then_inc(od_sem, 1)
                else:
                    tt_finish(g, lo, hi, BUF[(g, lo)])

    nc.compile()
    return nc


def _get_nc():
    if not hasattr(_cache, "nc"):
        _cache.nc = _build_nc()
    return _cache.nc


def _to_dr_layout(a_t):
    """[D, F] -> [DP_TILES, P, 2, F]: d = dp*256 + i*128 + p."""
    F = a_t.shape[1]
    return np.ascontiguousarray(
        a_t.reshape(DP_TILES, 2, P, F).transpose(0, 2, 1, 3)
    )


def kernel(inputs, centers, _trace=False):
    inputs = np.asarray(inputs, dtype=np.float32)
    centers = np.asarray(centers, dtype=np.float32)

    csq = np.sum(centers.astype(np.float64) ** 2, axis=1)
    xsq = np.sum(inputs.astype(np.float64) ** 2, axis=1)

    ct8 = _to_dr_layout(centers.T.astype(_NP_DT))
    xt8_full = (2.0 * inputs).T.astype(_NP_DT)  # [D, N]
    ncsq = np.ascontiguousarray((-csq).reshape(M_TILES, P).T.astype(np.float32))

    in_maps = []
    for i in range(N_CORES):
        sl = slice(i * NSH, (i + 1) * NSH)
        in_maps.append(
            {
                "ct": ct8,
                "xt": _to_dr_layout(xt8_full[:, sl]),
                "ncsq": ncsq,
                "nxsq": np.ascontiguousarray(
                    np.broadcast_to(
                        (-xsq[sl]).astype(np.float16), (P, NSH)
                    )
                ),
            }
        )

    nc = _get_nc()
    try:
        res = run_bass_kernel_spmd(
            nc, in_maps, core_ids=list(range(N_CORES)), trace=_trace
        )
    except ModuleNotFoundError:
        # NTFF trace glue is absent in some images; rerun without tracing
        res = run_bass_kernel_spmd(
            nc, in_maps, core_ids=list(range(N_CORES)), trace=False
        )
    if _trace:
        kernel.last_results = res
    return np.concatenate(
        [np.asarray(r["out"]).astype(np.float32) for r in res.results], axis=1
    )



# revision 45
# speedup vs baseline: 1.0581x; 1.0581x over previous
"""Trainium2 Bass kernel for nn_CentersDistance (retrieval_knn).

logits[k, n] = -||centers[k] - inputs[n]||^2
             = 2*(centers @ inputs.T)[k, n] - ||centers[k]||^2 - ||inputs[n]||^2

Strategy (8 NeuronCores, data-parallel over the N=8192 inputs):
  * host: transpose both operands so the contraction dim D lands on the SBUF
    partition axis, fold the factor 2 into the inputs, quantize both to
    fp8e4m3 (TRN float8e4), and precompute the norm terms exactly in float64.
  * device (per core): a 1024x1024x1024 matmul in fp8 with DoubleRow perf
    mode: each InstMatmult consumes TWO 128-deep contraction tiles laid out
    as [128, 2, free] (2 rows/cycle on the PE = 157 TF/s, 2x the bf16 rate),
    so the whole GEMM is 64 matmul instructions instead of 128.  PSUM
    accumulation stays fp32.
  * the PSUM->SBUF epilogue (add -||c||^2 per-partition and -||x||^2
    broadcast row, emit bf16) is the serial tail bottleneck (~740ns per
    [128, 512] group on the DVE), so it is split across THREE engines:
      - even groups: DVE scalar_tensor_tensor (one op per group);
      - odd groups: Activation engine adds the per-partition -||c||^2 via
        an Identity-activation bias read straight from PSUM (this also
        frees the PSUM bank for pass 2), then GpSimd adds the -||x||^2 row
        (tensor_add, SBUF-only - GpSimd cannot touch PSUM).
  * -||x||^2 is shipped as a single [1, 1024] fp32 row (4 KB) and broadcast
    to all 128 partitions on-chip by GpSimd (partition_broadcast), replacing
    the baseline's 512 KB host-broadcast load.
  * raw Block/semaphore implementation (not Tile), same skeleton as the
    bf16 baseline: two HW-DGE queues (Sync: xt, Scalar: ct) with one
    semaphore per d-pair; PE warmup matmuls bridge the NRT preamble until
    the first tile pair lands (~10.5us) and keep the HAM clock ramp alive
    (an idle PE resets it - observed: a 4us tile stall restarted the ramp
    and cost 13 matmuls at 1.2GHz); pass 1 (m-tiles 0-3) runs d outermost
    to pace with the streaming loads across 8 PSUM banks; pass 2 (m-tiles
    4-7) runs d innermost so each output group retires early and its
    epilogue + store overlap the remaining matmuls.
  * stores pair adjacent groups (same m-tile -> contiguous in out) into
    single [128, 1024] bf16 DMAs (2 KB/partition lines) alternating between
    the two queues; the last group is split in half across both queues to
    shorten the tail.  Output is bf16 (halves store traffic vs fp32; host
    converts back).

Accuracy: the exact f64 norm terms dominate the logits; fp8 cross term +
bf16 store measured absmax/scale 7.3e-3 vs the 2e-2 gate (bf16 baseline:
3.3e-4 at 45us, kept in kernel_bf16_baseline.py as fallback).

Measured on 8 axon-tunneled trn2 cores (NTFF): 30.5-32.9us across runs
(best 30490ns, median ~31.4us); shared-HBM/clock-throttle noise from
co-tenants can add up to +7us in bad windows.  Accuracy: absmax/scale
5.22e-3 vs the 2e-2 gate, bit-stable (fixed-seed inputs, deterministic
quantization and accumulation order).  bf16 fallback (3.3e-4, 45us) in
kernel_bf16_baseline.py.
"""

import threading
from contextlib import ExitStack

import numpy as np
import ml_dtypes

import concourse.mybir as mybir
from concourse import bacc
from concourse.bass_utils import run_bass_kernel_spmd

N_CORES = 8
N, K, D = 8192, 1024, 1024
NSH = N // N_CORES  # per-core slab of inputs
P = 128             # SBUF partitions
NF = 512            # matmul moving free dim (one fp32 PSUM bank)

DP_TILES = D // (2 * P)  # 4 double-row contraction tiles (256 deep each)
M_TILES = K // P         # 8 center tiles
H_TILES = NSH // NF      # 2 moving-dim tiles

G = M_TILES * H_TILES  # 16 output groups of [128, 512]
GP1 = 8                # groups 0-7 -> pass 1 (m-tiles 0-3), banks 0-7
N_WU = 14              # PE warm-up matmuls (bridge preamble-end -> first tile;
                       # spliced pre-barrier, so they start ~1.1us earlier
                       # than the engine body and need ~4 extra to cover)
N_TMP = 6              # fp16 staging buffers for the Act->DVE drain pipeline

_DT = mybir.dt.float8e4
_NP_DT = ml_dtypes.float8_e4m3
_OUT_DT = mybir.dt.float16
_DR = mybir.MatmulPerfMode.DoubleRow

_cache = threading.local()


def _g_mh(g):
    return g // H_TILES, g % H_TILES


def _build_nc():
    nc = bacc.Bacc(
        "TRN2", target_bir_lowering=False, debug=False, num_devices=N_CORES
    )
    ct = nc.dram_tensor("ct", [DP_TILES, P, 2, K], _DT, kind="ExternalInput").ap()
    xt = nc.dram_tensor("xt", [DP_TILES, P, 2, NSH], _DT, kind="ExternalInput").ap()
    ncsq = nc.dram_tensor(
        "ncsq", [P, M_TILES], mybir.dt.float32, kind="ExternalInput"
    ).ap()
    nxsq = nc.dram_tensor(
        "nxsq", [P, NSH], mybir.dt.float16, kind="ExternalInput"
    ).ap()
    out = nc.dram_tensor("out", [K, NSH], _OUT_DT, kind="ExternalOutput").ap()

    out_r = out.rearrange("(m p) n -> m p n", p=P)

    HNF = NF // 2

    # Epilogue schedule.  DIRECT groups: one DVE scalar_tensor_tensor from
    # PSUM.  Others: Act drains (ps + ncsq -> tmp fp16), DVE finishes
    # (tmp + nxsq -> ot).  Group 15 is processed in two halves for tail
    # latency.  DVE processes groups in order, so od_sem == number of
    # groups (items) whose ot_sb data is final - stores key off it.
    DIRECT = {0, 4, G - 2, G - 1}
    items = [(g, 0, NF) for g in range(G - 1)]
    items += [(G - 1, 0, HNF), (G - 1, HNF, NF)]
    DVE_LIST = items
    ACT_LIST = [it for it in items if it[0] not in DIRECT]

    def _item_glh(item):
        return item

    OB = {}   # (g, lo) -> ob_sem count after its Act drain
    BUF = {}  # (g, lo) -> tmp buffer index
    for j, (g, lo, hi) in enumerate(ACT_LIST):
        OB[(g, lo)] = j + 1
        BUF[(g, lo)] = j % N_TMP
    FIN_OD = {}  # item -> od count after its DVE finish
    for j, it in enumerate(DVE_LIST):
        FIN_OD[it] = j + 1
    # pass-2 bank reuse: who releases bank g (the engine that READS PSUM)
    DRAIN_DONE = {}
    for g in range(GP1):
        if g in DIRECT:
            DRAIN_DONE[g] = ("od", FIN_OD[(g, 0, NF)])
        else:
            DRAIN_DONE[g] = ("ob", OB[(g, 0)])

    with (
        nc.sbuf_tensor("wu_sb", [P, 2, NF], _DT) as wu_sb,
        nc.sbuf_tensor("ncsq_sb", [P, M_TILES], mybir.dt.float32) as ncsq_sb,
        nc.sbuf_tensor("nxsq_sb", [P, NSH], mybir.dt.float16) as nxsq_sb,
        nc.sbuf_tensor("tmp_sb", [P, N_TMP, NF], mybir.dt.float16) as tmp_sb,
        nc.sbuf_tensor("ot_sb", [P, G * NF], _OUT_DT) as ot_sb,
        ExitStack() as stack,
        nc.semaphore("const_sem") as const_sem,
        nc.semaphore("bc_sem") as bc_sem,
        nc.semaphore("mm_sem") as mm_sem,
        nc.semaphore("ob_sem") as ob_sem, # group PSUM drained (Act)
        nc.semaphore("od_sem") as od_sem, # group data ready in ot_sb (DVE)
        nc.semaphore("ds_sync") as ds_sync,
        nc.semaphore("ds_ring") as ds_ring,
        nc.semaphore("ds_scalar") as ds_scalar,
        nc.Block() as block,
    ):
        d_sems = [
            stack.enter_context(nc.semaphore(f"d_sem{i}")) for i in range(DP_TILES)
        ]
        ct_sb = [
            stack.enter_context(nc.sbuf_tensor(f"ct_sb{d}", [P, 2, K], _DT))
            for d in range(DP_TILES)
        ]
        xt_sb = [
            stack.enter_context(nc.sbuf_tensor(f"xt_sb{d}", [P, 2, NSH], _DT))
            for d in range(DP_TILES)
        ]
        ps = [
            stack.enter_context(nc.psum_tensor(f"ps{b}", [P, NF], mybir.dt.float32))
            for b in range(8)
        ]

        # Warmups are emitted here (current block is still 'main') and
        # spliced after the PE's entry-barrier arrive (InstDrain) below,
        # so the HAM clock ramp starts ~1.1us before the barrier resolves
        # and the real matmuls run fully warm.  No DMA program is touched:
        # reordering the load dispatches measurably perturbs the DMA
        # fabric arbitration against us (v7/v8 negative results), so the
        # entire load schedule stays byte-identical to the baseline.
        wu_insts = []
        for _ in range(N_WU):
            wu_insts.append(
                nc.tensor.matmul(
                    ps[GP1 - 1][:],
                    wu_sb[:, :, 0:P],
                    wu_sb[:, :, :],
                    start=True,
                    stop=True,
                    perf_mode=_DR,
                ).ins
            )

        # store schedule: pair adjacent groups (same m-tile -> contiguous
        # in out) into one [128, 1024] fp16 DMA with 2KB/partition lines.
        # Sync queue: pairs 0,2,4,6 + group 14 + first half of group 15.
        # GpSimd SWDGE ring: pairs 1,3,5 + second half of group 15 (the Act
        # engine dispatches no stores at all - it is the single PSUM
        # drainer and must not stall behind store waits).
        def pair_store(eng, gp, sem):
            g0 = 2 * gp
            m, _ = _g_mh(g0)
            eng.wait_ge(od_sem, g0 + 2)
            eng.dma_start(
                out_r[m][:],
                ot_sb[:, g0 * NF : (g0 + 2) * NF],
            ).then_inc(sem, 16)

        @block.sync
        def _(sync):
            # ncsq first: 4KB, needed by the Act drains
            sync.dma_start(ncsq_sb[:], ncsq).then_inc(const_sem, 16)
            for d in (0, 2, 3):
                sync.dma_start(xt_sb[d][:], xt[d]).then_inc(d_sems[d], 16)
            for gp in (0, 2, 4):
                pair_store(sync, gp, ds_sync)
            # group 14 whole
            m, h = _g_mh(G - 2)
            sync.wait_ge(od_sem, G - 1)
            sync.dma_start(
                out_r[m][:, h * NF : (h + 1) * NF],
                ot_sb[:, (G - 2) * NF : (G - 1) * NF],
            ).then_inc(ds_sync, 16)
            # first half of group 15
            m, h = _g_mh(G - 1)
            sync.wait_ge(od_sem, G)
            sync.dma_start(
                out_r[m][:, h * NF : h * NF + HNF],
                ot_sb[:, (G - 1) * NF : (G - 1) * NF + HNF],
            ).then_inc(ds_sync, 16)
            # wait for THIS queue's stores only; scalar and gpsimd wait for
            # theirs in parallel, so the end-of-kernel barrier sees the max,
            # not the sum of completion-semaphore latencies
            sync.wait_ge(ds_sync, 5 * 16)

        @block.scalar
        def _(scalar):
            for d in (0, 2, 3):
                scalar.dma_start(ct_sb[d][:], ct[d]).then_inc(d_sems[d], 16)
            # Act drains the non-DIRECT groups: tmp = Identity(ps +
            # ncsq_bias), fp32 PSUM -> fp16 SBUF, ~720ns measured per
            # [128, 512] group.  DIRECT groups go via a single DVE
            # scalar_tensor_tensor instead (745ns) - the 4/12 split
            # balances the two engines at ~8.5us each.
            scalar.wait_ge(const_sem, 16)
            n_act = 0

            def act_drain(g, lo, hi):
                nonlocal n_act
                buf = n_act % N_TMP
                if n_act >= N_TMP:
                    # tmp reuse: the DVE finish for this buffer's previous
                    # occupant must have consumed it
                    scalar.wait_ge(od_sem, FIN_OD[ACT_LIST[n_act - N_TMP]])
                m, _ = _g_mh(g)
                nc.scalar.add(
                    tmp_sb[:, buf, 0 : hi - lo],
                    ps[g % 8][:, lo:hi],
                    ncsq_sb[:, m : m + 1],
                ).then_inc(ob_sem, 1)
                n_act += 1

            for item in ACT_LIST:
                g, lo, hi = _item_glh(item)
                scalar.wait_ge(mm_sem, min(g + 1, G))
                act_drain(g, lo, hi)
            # late stores from here: the Act engine is idle after its last
            # drain and dispatches fast
            pair_store(scalar, 6, ds_scalar)
            m, h = _g_mh(G - 1)
            scalar.wait_ge(od_sem, G + 1)
            scalar.dma_start(
                out_r[m][:, h * NF + HNF : (h + 1) * NF],
                ot_sb[:, (G - 1) * NF + HNF : G * NF],
            ).then_inc(ds_scalar, 16)
            scalar.wait_ge(ds_scalar, 2 * 16)

        @block.gpsimd
        def _(gpsimd):
            # dp1 rides the software-DGE ring: it spins up ~2us after the HW
            # queues and lands its pair second, right when pass 1 wants it.
            # GpSimd does DMA dispatch ONLY (its partition_broadcast ucode
            # carries a pool-reconfig DRAIN that blocked the engine ~12us),
            # so nxsq ships host-broadcast (256KB fp16) as the third stream.
            gpsimd.dma_start(xt_sb[1][:], xt[1]).then_inc(d_sems[1], 16)
            gpsimd.dma_start(ct_sb[1][:], ct[1]).then_inc(d_sems[1], 16)
            gpsimd.dma_start(nxsq_sb[:], nxsq).then_inc(bc_sem, 16)
            # odd pair stores ride the ring too
            for gp in (1, 3, 5):
                pair_store(gpsimd, gp, ds_ring)
            gpsimd.wait_ge(ds_ring, 3 * 16)

        @block.tensor
        def _(tensor):
            # warm-ups are spliced pre-barrier (see above); wu_sb is
            # deliberately uninitialized - the products are never read.
            # Bank 7 is rewritten with start=True by group 7's first matmul.
            # pass 1: groups 0-7 accumulate in banks 0-7, d outermost so
            # matmuls pace with the streaming loads
            for d in range(DP_TILES):
                tensor.wait_ge(d_sems[d], 32)
                for g in range(GP1):
                    m, h = _g_mh(g)
                    mm = nc.tensor.matmul(
                        ps[g][:],
                        ct_sb[d][:, :, m * P : (m + 1) * P],
                        xt_sb[d][:, :, h * NF : (h + 1) * NF],
                        start=(d == 0),
                        stop=(d == DP_TILES - 1),
                        perf_mode=_DR,
                    )
                    if d == DP_TILES - 1:
                        mm.then_inc(mm_sem, 1)
            # pass 2: groups 8-15 reuse banks 0-7 once the Act engine has
            # drained the pass-1 group from that bank (P10: concurrent
            # PE-write + engine-read of one PSUM bank is fatal, so this wait
            # is load-bearing, not just WAR ordering)
            for g in range(GP1, G):
                m, h = _g_mh(g)
                dsem, dcnt = DRAIN_DONE[g - 8]
                tensor.wait_ge(ob_sem if dsem == "ob" else od_sem, dcnt)
                for d in range(DP_TILES):
                    mm = nc.tensor.matmul(
                        ps[g % 8][:],
                        ct_sb[d][:, :, m * P : (m + 1) * P],
                        xt_sb[d][:, :, h * NF : (h + 1) * NF],
                        start=(d == 0),
                        stop=(d == DP_TILES - 1),
                        perf_mode=_DR,
                    )
                mm.then_inc(mm_sem, 1)

        @block.vector
        def _(vector):
            vector.wait_ge(bc_sem, 16)     # nxsq landed
            vector.wait_ge(const_sem, 16)  # ncsq (for the direct STTs)

            def tt_finish(g, lo, hi, buf):
                # finish: ot = tmp + nxsq.  All-fp16 SBUF operands -> DVE
                # 2x perf mode (~423ns vs 745ns for the PSUM STT).
                vector.wait_ge(ob_sem, OB[(g, lo)])
                _, h = _g_mh(g)
                nc.vector.tensor_add(
                    ot_sb[:, g * NF + lo : g * NF + hi],
                    tmp_sb[:, buf, 0 : hi - lo],
                    nxsq_sb[:, h * NF + lo : h * NF + hi],
                ).then_inc(od_sem, 1)

            for item in DVE_LIST:
                g, lo, hi = _item_glh(item)
                if g in DIRECT:
                    m, h = _g_mh(g)
                    vector.wait_ge(mm_sem, min(g + 1, G))
                    nc.vector.scalar_tensor_tensor(
                        ot_sb[:, g * NF + lo : g * NF + hi],
                        ps[g % 8][:, lo:hi],
                        ncsq_sb[:, m : m + 1],
                        nxsq_sb[:, h * NF + lo : h * NF + hi],
                        op0=mybir.AluOpType.add,
                        op1=mybir.AluOpType.add,
                    ).then_inc(od_sem, 1)
                else:
                    tt_finish(g, lo, hi, BUF[(g, lo)])

        # splice: move the warmups between the PE's entry-barrier arrive
        # (InstDrain) and its barrier-wait EVENT_SEMAPHORE in 'main'.
        try:
            entry = nc.main_func.blocks[0]
            il = entry.instructions
            for inst in wu_insts:
                il.remove(inst)
            pos = (
                next(
                    i
                    for i, x in enumerate(il)
                    if isinstance(x, mybir.InstDrain)
                    and x.engine == mybir.EngineType.PE
                )
                + 1
            )
            for k, inst in enumerate(wu_insts):
                il.insert(pos + k, inst)
        except Exception:
            pass  # fall back to post-barrier placement (still correct)

    nc.compile()
    return nc


def _get_nc():
    if not hasattr(_cache, "nc"):
        _cache.nc = _build_nc()
    return _cache.nc


def _to_dr_layout(a_t):
    """[D, F] -> [DP_TILES, P, 2, F]: d = dp*256 + i*128 + p."""
    F = a_t.shape[1]
    return np.ascontiguousarray(
        a_t.reshape(DP_TILES, 2, P, F).transpose(0, 2, 1, 3)
    )


def kernel(inputs, centers, _trace=False):
    inputs = np.asarray(inputs, dtype=np.float32)
    centers = np.asarray(centers, dtype=np.float32)

    csq = np.sum(centers.astype(np.float64) ** 2, axis=1)
    xsq = np.sum(inputs.astype(np.float64) ** 2, axis=1)

    ct8 = _to_dr_layout(centers.T.astype(_NP_DT))
    xt8_full = (2.0 * inputs).T.astype(_NP_DT)  # [D, N]
    ncsq = np.ascontiguousarray((-csq).reshape(M_TILES, P).T.astype(np.float32))

    in_maps = []
    for i in range(N_CORES):
        sl = slice(i * NSH, (i + 1) * NSH)
        in_maps.append(
            {
                "ct": ct8,
                "xt": _to_dr_layout(xt8_full[:, sl]),
                "ncsq": ncsq,
                "nxsq": np.ascontiguousarray(
                    np.broadcast_to(
                        (-xsq[sl]).astype(np.float16), (P, NSH)
                    )
                ),
            }
        )

    nc = _get_nc()
    try:
        res = run_bass_kernel_spmd(
            nc, in_maps, core_ids=list(range(N_CORES)), trace=_trace
        )
    except ModuleNotFoundError:
        # NTFF trace glue is absent in some images; rerun without tracing
        res = run_bass_kernel_spmd(
            nc, in_maps, core_ids=list(range(N_CORES)), trace=False
        )
    if _trace:
        kernel.last_results = res
    return np.concatenate(
        [np.asarray(r["out"]).astype(np.float32) for r in res.results], axis=1
    )



# revision 46
# speedup vs baseline: 1.0838x; 1.0243x over previous
"""Trainium2 Bass kernel for nn_CentersDistance (retrieval_knn).

logits[k, n] = -||centers[k] - inputs[n]||^2
             = 2*(centers @ inputs.T)[k, n] - ||centers[k]||^2 - ||inputs[n]||^2

Strategy (8 NeuronCores, data-parallel over the N=8192 inputs):
  * host: transpose both operands so the contraction dim D lands on the SBUF
    partition axis, fold the factor 2 into the inputs, quantize both to
    fp8e4m3 (TRN float8e4), and precompute the norm terms exactly in float64.
  * device (per core): a 1024x1024x1024 matmul in fp8 with DoubleRow perf
    mode: each InstMatmult consumes TWO 128-deep contraction tiles laid out
    as [128, 2, free] (2 rows/cycle on the PE = 157 TF/s, 2x the bf16 rate),
    so the whole GEMM is 64 matmul instructions instead of 128.  PSUM
    accumulation stays fp32.
  * the PSUM->SBUF epilogue (add -||c||^2 per-partition and -||x||^2
    broadcast row, emit bf16) is the serial tail bottleneck (~740ns per
    [128, 512] group on the DVE), so it is split across THREE engines:
      - even groups: DVE scalar_tensor_tensor (one op per group);
      - odd groups: Activation engine adds the per-partition -||c||^2 via
        an Identity-activation bias read straight from PSUM (this also
        frees the PSUM bank for pass 2), then GpSimd adds the -||x||^2 row
        (tensor_add, SBUF-only - GpSimd cannot touch PSUM).
  * -||x||^2 is shipped as a single [1, 1024] fp32 row (4 KB) and broadcast
    to all 128 partitions on-chip by GpSimd (partition_broadcast), replacing
    the baseline's 512 KB host-broadcast load.
  * raw Block/semaphore implementation (not Tile), same skeleton as the
    bf16 baseline: two HW-DGE queues (Sync: xt, Scalar: ct) with one
    semaphore per d-pair; PE warmup matmuls bridge the NRT preamble until
    the first tile pair lands (~10.5us) and keep the HAM clock ramp alive
    (an idle PE resets it - observed: a 4us tile stall restarted the ramp
    and cost 13 matmuls at 1.2GHz); pass 1 (m-tiles 0-3) runs d outermost
    to pace with the streaming loads across 8 PSUM banks; pass 2 (m-tiles
    4-7) runs d innermost so each output group retires early and its
    epilogue + store overlap the remaining matmuls.
  * stores pair adjacent groups (same m-tile -> contiguous in out) into
    single [128, 1024] bf16 DMAs (2 KB/partition lines) alternating between
    the two queues; the last group is split in half across both queues to
    shorten the tail.  Output is bf16 (halves store traffic vs fp32; host
    converts back).

Accuracy: the exact f64 norm terms dominate the logits; fp8 cross term +
bf16 store measured absmax/scale 7.3e-3 vs the 2e-2 gate (bf16 baseline:
3.3e-4 at 45us, kept in kernel_bf16_baseline.py as fallback).

Measured on 8 axon-tunneled trn2 cores (NTFF): 30.5-32.9us across runs
(best 30490ns, median ~31.4us); shared-HBM/clock-throttle noise from
co-tenants can add up to +7us in bad windows.  Accuracy: absmax/scale
5.22e-3 vs the 2e-2 gate, bit-stable (fixed-seed inputs, deterministic
quantization and accumulation order).  bf16 fallback (3.3e-4, 45us) in
kernel_bf16_baseline.py.
"""

import threading
from contextlib import ExitStack

import numpy as np
import ml_dtypes

import concourse.mybir as mybir
from concourse import bacc
from concourse.bass_utils import run_bass_kernel_spmd

N_CORES = 8
N, K, D = 8192, 1024, 1024
NSH = N // N_CORES  # per-core slab of inputs
P = 128             # SBUF partitions
NF = 512            # matmul moving free dim (one fp32 PSUM bank)

DP_TILES = D // (2 * P)  # 4 double-row contraction tiles (256 deep each)
M_TILES = K // P         # 8 center tiles
H_TILES = NSH // NF      # 2 moving-dim tiles

G = M_TILES * H_TILES  # 16 output groups of [128, 512]
GP1 = 8                # groups 0-7 -> pass 1 (m-tiles 0-3), banks 0-7
N_WU = 12              # PE warm-up matmuls (bridge preamble-end -> first tile;
                       # spliced pre-barrier ~6.5us, 427ns each -> ends ~11.7,
                       # just short of the earliest observed d0 landing
                       # (~12.0): queued warmups PAST data-arrival delay the
                       # first real matmul (measured 0.33-0.39us waste at
                       # N_WU=14 in both v9 traces), while undershooting
                       # only idles the PE (1-2us idle << the 3.4us HAM
                       # window, no re-throttle)
N_TMP = 6              # fp16 staging buffers for the Act->DVE drain pipeline

_DT = mybir.dt.float8e4
_NP_DT = ml_dtypes.float8_e4m3
_OUT_DT = mybir.dt.float16
_DR = mybir.MatmulPerfMode.DoubleRow

_cache = threading.local()


def _g_mh(g):
    return g // H_TILES, g % H_TILES


def _build_nc():
    nc = bacc.Bacc(
        "TRN2", target_bir_lowering=False, debug=False, num_devices=N_CORES
    )
    ct = nc.dram_tensor("ct", [DP_TILES, P, 2, K], _DT, kind="ExternalInput").ap()
    xt = nc.dram_tensor("xt", [DP_TILES, P, 2, NSH], _DT, kind="ExternalInput").ap()
    ncsq = nc.dram_tensor(
        "ncsq", [P, M_TILES], mybir.dt.float32, kind="ExternalInput"
    ).ap()
    nxsq = nc.dram_tensor(
        "nxsq", [P, NSH], mybir.dt.float16, kind="ExternalInput"
    ).ap()
    out = nc.dram_tensor("out", [K, NSH], _OUT_DT, kind="ExternalOutput").ap()

    out_r = out.rearrange("(m p) n -> m p n", p=P)

    HNF = NF // 2

    # Epilogue schedule.  DIRECT groups: one DVE scalar_tensor_tensor from
    # PSUM.  Others: Act drains (ps + ncsq -> tmp fp16), DVE finishes
    # (tmp + nxsq -> ot).  Group 15 is processed in two halves for tail
    # latency.  DVE processes groups in order, so od_sem == number of
    # groups (items) whose ot_sb data is final - stores key off it.
    DIRECT = {0, 4, G - 2, G - 1}
    items = [(g, 0, NF) for g in range(G - 1)]
    items += [(G - 1, 0, HNF), (G - 1, HNF, NF)]
    DVE_LIST = items
    ACT_LIST = [it for it in items if it[0] not in DIRECT]

    def _item_glh(item):
        return item

    OB = {}   # (g, lo) -> ob_sem count after its Act drain
    BUF = {}  # (g, lo) -> tmp buffer index
    for j, (g, lo, hi) in enumerate(ACT_LIST):
        OB[(g, lo)] = j + 1
        BUF[(g, lo)] = j % N_TMP
    FIN_OD = {}  # item -> od count after its DVE finish
    for j, it in enumerate(DVE_LIST):
        FIN_OD[it] = j + 1
    # pass-2 bank reuse: who releases bank g (the engine that READS PSUM)
    DRAIN_DONE = {}
    for g in range(GP1):
        if g in DIRECT:
            DRAIN_DONE[g] = ("od", FIN_OD[(g, 0, NF)])
        else:
            DRAIN_DONE[g] = ("ob", OB[(g, 0)])

    with (
        nc.sbuf_tensor("wu_sb", [P, 2, NF], _DT) as wu_sb,
        nc.sbuf_tensor("ncsq_sb", [P, M_TILES], mybir.dt.float32) as ncsq_sb,
        nc.sbuf_tensor("nxsq_sb", [P, NSH], mybir.dt.float16) as nxsq_sb,
        nc.sbuf_tensor("tmp_sb", [P, N_TMP, NF], mybir.dt.float16) as tmp_sb,
        nc.sbuf_tensor("ot_sb", [P, G * NF], _OUT_DT) as ot_sb,
        ExitStack() as stack,
        nc.semaphore("const_sem") as const_sem,
        nc.semaphore("bc_sem") as bc_sem,
        nc.semaphore("mm_sem") as mm_sem,
        nc.semaphore("ob_sem") as ob_sem, # group PSUM drained (Act)
        nc.semaphore("od_sem") as od_sem, # group data ready in ot_sb (DVE)
        nc.semaphore("ds_sync") as ds_sync,
        nc.semaphore("ds_ring") as ds_ring,
        nc.semaphore("ds_scalar") as ds_scalar,
        nc.Block() as block,
    ):
        d_sems = [
            stack.enter_context(nc.semaphore(f"d_sem{i}")) for i in range(DP_TILES)
        ]
        ct_sb = [
            stack.enter_context(nc.sbuf_tensor(f"ct_sb{d}", [P, 2, K], _DT))
            for d in range(DP_TILES)
        ]
        xt_sb = [
            stack.enter_context(nc.sbuf_tensor(f"xt_sb{d}", [P, 2, NSH], _DT))
            for d in range(DP_TILES)
        ]
        ps = [
            stack.enter_context(nc.psum_tensor(f"ps{b}", [P, NF], mybir.dt.float32))
            for b in range(8)
        ]

        # Warmups are emitted here (current block is still 'main') and
        # spliced after the PE's entry-barrier arrive (InstDrain) below,
        # so the HAM clock ramp starts ~1.1us before the barrier resolves
        # and the real matmuls run fully warm.  No DMA program is touched:
        # reordering the load dispatches measurably perturbs the DMA
        # fabric arbitration against us (v7/v8 negative results), so the
        # entire load schedule stays byte-identical to the baseline.
        wu_insts = []
        for _ in range(N_WU):
            wu_insts.append(
                nc.tensor.matmul(
                    ps[GP1 - 1][:],
                    wu_sb[:, :, 0:P],
                    wu_sb[:, :, :],
                    start=True,
                    stop=True,
                    perf_mode=_DR,
                ).ins
            )

        # store schedule: pair adjacent groups (same m-tile -> contiguous
        # in out) into one [128, 1024] fp16 DMA with 2KB/partition lines.
        # Sync queue: pairs 0,2,4,6 + group 14 + first half of group 15.
        # GpSimd SWDGE ring: pairs 1,3,5 + second half of group 15 (the Act
        # engine dispatches no stores at all - it is the single PSUM
        # drainer and must not stall behind store waits).
        def pair_store(eng, gp, sem):
            g0 = 2 * gp
            m, _ = _g_mh(g0)
            eng.wait_ge(od_sem, g0 + 2)
            eng.dma_start(
                out_r[m][:],
                ot_sb[:, g0 * NF : (g0 + 2) * NF],
            ).then_inc(sem, 16)

        @block.sync
        def _(sync):
            # ncsq first: 4KB, needed by the Act drains
            sync.dma_start(ncsq_sb[:], ncsq).then_inc(const_sem, 16)
            for d in (0, 2, 3):
                sync.dma_start(xt_sb[d][:], xt[d]).then_inc(d_sems[d], 16)
            for gp in (0, 2, 4):
                pair_store(sync, gp, ds_sync)
            # group 14 whole
            m, h = _g_mh(G - 2)
            sync.wait_ge(od_sem, G - 1)
            sync.dma_start(
                out_r[m][:, h * NF : (h + 1) * NF],
                ot_sb[:, (G - 2) * NF : (G - 1) * NF],
            ).then_inc(ds_sync, 16)
            # first half of group 15
            m, h = _g_mh(G - 1)
            sync.wait_ge(od_sem, G)
            sync.dma_start(
                out_r[m][:, h * NF : h * NF + HNF],
                ot_sb[:, (G - 1) * NF : (G - 1) * NF + HNF],
            ).then_inc(ds_sync, 16)
            # wait for THIS queue's stores only; scalar and gpsimd wait for
            # theirs in parallel, so the end-of-kernel barrier sees the max,
            # not the sum of completion-semaphore latencies
            sync.wait_ge(ds_sync, 5 * 16)

        @block.scalar
        def _(scalar):
            for d in (0, 2, 3):
                scalar.dma_start(ct_sb[d][:], ct[d]).then_inc(d_sems[d], 16)
            # Act drains the non-DIRECT groups: tmp = Identity(ps +
            # ncsq_bias), fp32 PSUM -> fp16 SBUF, ~720ns measured per
            # [128, 512] group.  DIRECT groups go via a single DVE
            # scalar_tensor_tensor instead (745ns) - the 4/12 split
            # balances the two engines at ~8.5us each.
            scalar.wait_ge(const_sem, 16)
            n_act = 0

            def act_drain(g, lo, hi):
                nonlocal n_act
                buf = n_act % N_TMP
                if n_act >= N_TMP:
                    # tmp reuse: the DVE finish for this buffer's previous
                    # occupant must have consumed it
                    scalar.wait_ge(od_sem, FIN_OD[ACT_LIST[n_act - N_TMP]])
                m, _ = _g_mh(g)
                nc.scalar.add(
                    tmp_sb[:, buf, 0 : hi - lo],
                    ps[g % 8][:, lo:hi],
                    ncsq_sb[:, m : m + 1],
                ).then_inc(ob_sem, 1)
                n_act += 1

            for item in ACT_LIST:
                g, lo, hi = _item_glh(item)
                scalar.wait_ge(mm_sem, min(g + 1, G))
                act_drain(g, lo, hi)
            # late stores from here: the Act engine is idle after its last
            # drain and dispatches fast
            pair_store(scalar, 6, ds_scalar)
            m, h = _g_mh(G - 1)
            scalar.wait_ge(od_sem, G + 1)
            scalar.dma_start(
                out_r[m][:, h * NF + HNF : (h + 1) * NF],
                ot_sb[:, (G - 1) * NF + HNF : G * NF],
            ).then_inc(ds_scalar, 16)
            scalar.wait_ge(ds_scalar, 2 * 16)

        @block.gpsimd
        def _(gpsimd):
            # dp1 rides the software-DGE ring: it spins up ~2us after the HW
            # queues and lands its pair second, right when pass 1 wants it.
            # GpSimd does DMA dispatch ONLY (its partition_broadcast ucode
            # carries a pool-reconfig DRAIN that blocked the engine ~12us),
            # so nxsq ships host-broadcast (256KB fp16) as the third stream.
            gpsimd.dma_start(xt_sb[1][:], xt[1]).then_inc(d_sems[1], 16)
            gpsimd.dma_start(ct_sb[1][:], ct[1]).then_inc(d_sems[1], 16)
            gpsimd.dma_start(nxsq_sb[:], nxsq).then_inc(bc_sem, 16)
            # odd pair stores ride the ring too
            for gp in (1, 3, 5):
                pair_store(gpsimd, gp, ds_ring)
            gpsimd.wait_ge(ds_ring, 3 * 16)

        @block.tensor
        def _(tensor):
            # warm-ups are spliced pre-barrier (see above); wu_sb is
            # deliberately uninitialized - the products are never read.
            # Bank 7 is rewritten with start=True by group 7's first matmul.
            # pass 1: groups 0-7 accumulate in banks 0-7, d outermost so
            # matmuls pace with the streaming loads
            for d in range(DP_TILES):
                tensor.wait_ge(d_sems[d], 32)
                for g in range(GP1):
                    m, h = _g_mh(g)
                    mm = nc.tensor.matmul(
                        ps[g][:],
                        ct_sb[d][:, :, m * P : (m + 1) * P],
                        xt_sb[d][:, :, h * NF : (h + 1) * NF],
                        start=(d == 0),
                        stop=(d == DP_TILES - 1),
                        perf_mode=_DR,
                    )
                    if d == DP_TILES - 1:
                        mm.then_inc(mm_sem, 1)
            # pass 2: groups 8-15 reuse banks 0-7 once the Act engine has
            # drained the pass-1 group from that bank (P10: concurrent
            # PE-write + engine-read of one PSUM bank is fatal, so this wait
            # is load-bearing, not just WAR ordering)
            for g in range(GP1, G):
                m, h = _g_mh(g)
                dsem, dcnt = DRAIN_DONE[g - 8]
                tensor.wait_ge(ob_sem if dsem == "ob" else od_sem, dcnt)
                for d in range(DP_TILES):
                    mm = nc.tensor.matmul(
                        ps[g % 8][:],
                        ct_sb[d][:, :, m * P : (m + 1) * P],
                        xt_sb[d][:, :, h * NF : (h + 1) * NF],
                        start=(d == 0),
                        stop=(d == DP_TILES - 1),
                        perf_mode=_DR,
                    )
                mm.then_inc(mm_sem, 1)

        @block.vector
        def _(vector):
            vector.wait_ge(bc_sem, 16)     # nxsq landed
            vector.wait_ge(const_sem, 16)  # ncsq (for the direct STTs)

            def tt_finish(g, lo, hi, buf):
                # finish: ot = tmp + nxsq.  All-fp16 SBUF operands -> DVE
                # 2x perf mode (~423ns vs 745ns for the PSUM STT).
                vector.wait_ge(ob_sem, OB[(g, lo)])
                _, h = _g_mh(g)
                nc.vector.tensor_add(
                    ot_sb[:, g * NF + lo : g * NF + hi],
                    tmp_sb[:, buf, 0 : hi - lo],
                    nxsq_sb[:, h * NF + lo : h * NF + hi],
                ).then_inc(od_sem, 1)

            for item in DVE_LIST:
                g, lo, hi = _item_glh(item)
                if g in DIRECT:
                    m, h = _g_mh(g)
                    vector.wait_ge(mm_sem, min(g + 1, G))
                    nc.vector.scalar_tensor_tensor(
                        ot_sb[:, g * NF + lo : g * NF + hi],
                        ps[g % 8][:, lo:hi],
                        ncsq_sb[:, m : m + 1],
                        nxsq_sb[:, h * NF + lo : h * NF + hi],
                        op0=mybir.AluOpType.add,
                        op1=mybir.AluOpType.add,
                    ).then_inc(od_sem, 1)
                else:
                    tt_finish(g, lo, hi, BUF[(g, lo)])

        # splice: move the warmups between the PE's entry-barrier arrive
        # (InstDrain) and its barrier-wait EVENT_SEMAPHORE in 'main'.
        try:
            entry = nc.main_func.blocks[0]
            il = entry.instructions
            for inst in wu_insts:
                il.remove(inst)
            pos = (
                next(
                    i
                    for i, x in enumerate(il)
                    if isinstance(x, mybir.InstDrain)
                    and x.engine == mybir.EngineType.PE
                )
                + 1
            )
            for k, inst in enumerate(wu_insts):
                il.insert(pos + k, inst)
        except Exception:
            pass  # fall back to post-barrier placement (still correct)

    nc.compile()
    return nc


def _get_nc():
    if not hasattr(_cache, "nc"):
        _cache.nc = _build_nc()
    return _cache.nc


def _to_dr_layout(a_t):
    """[D, F] -> [DP_TILES, P, 2, F]: d = dp*256 + i*128 + p."""
    F = a_t.shape[1]
    return np.ascontiguousarray(
        a_t.reshape(DP_TILES, 2, P, F).transpose(0, 2, 1, 3)
    )


def kernel(inputs, centers, _trace=False):
    inputs = np.asarray(inputs, dtype=np.float32)
    centers = np.asarray(centers, dtype=np.float32)

    csq = np.sum(centers.astype(np.float64) ** 2, axis=1)
    xsq = np.sum(inputs.astype(np.float64) ** 2, axis=1)

    ct8 = _to_dr_layout(centers.T.astype(_NP_DT))
    xt8_full = (2.0 * inputs).T.astype(_NP_DT)  # [D, N]
    ncsq = np.ascontiguousarray((-csq).reshape(M_TILES, P).T.astype(np.float32))

    in_maps = []
    for i in range(N_CORES):
        sl = slice(i * NSH, (i + 1) * NSH)
        in_maps.append(
            {
                "ct": ct8,
                "xt": _to_dr_layout(xt8_full[:, sl]),
                "ncsq": ncsq,
                "nxsq": np.ascontiguousarray(
                    np.broadcast_to(
                        (-xsq[sl]).astype(np.float16), (P, NSH)
                    )
                ),
            }
        )

    nc = _get_nc()
    try:
        res = run_bass_kernel_spmd(
            nc, in_maps, core_ids=list(range(N_CORES)), trace=_trace
        )
    except ModuleNotFoundError:
        # NTFF trace glue is absent in some images; rerun without tracing
        res = run_bass_kernel_spmd(
            nc, in_maps, core_ids=list(range(N_CORES)), trace=False
        )
    if _trace:
        kernel.last_results = res
    return np.concatenate(
        [np.asarray(r["out"]).astype(np.float32) for r in res.results], axis=1
    )

